# revision 29
# baseline (speedup 1.0000x reference)
"""DETR scene-graph predicate head on 8 Trainium2 NeuronCores.

Math: logits[l,b,r,:] = concat(hs[l,b,q_sub], hs[l,b,q_obj]) @ W_pred.T + b_pred
where q_sub/q_obj come from (tgt_perm inverse, relationships, src_indices) —
pure integer index math, done on host.

Key factorization: split W_pred [P, 2D] into W_sub|W_obj [P, D] halves and
compute, per (layer, image) block, U = matched @ W_sub.T and V = matched @
W_obj.T over the M=64 *matched* queries (matched = hs[l,b,src_indices[l,b]]).
Then logits[r] = U[pos_sub[r]] + V[pos_obj[r]] + b — a cheap index-select the
host applies while unsharding. This cuts device HBM traffic 3x vs shipping
hs+one-hots (each matched row is read once, not once per relation end) and
turns the whole kernel into one W-stationary GEMM.

Device layout (batch axis sharded 8 ways; L*B/8 = 192 blocks/core):
  - Host sends matched.T per block as two 128-row chunks [2, 128, 64] bf16,
    packed 8 blocks/group side-by-side -> rhs tiles [128, 512], two groups per
    DMA [128, 2048] (4 KB/partition lines keep all 16 SDMA engines busy on the
    gpsimd SWDGE queue).
  - Per group: 2 accumulating matmuls with stationary W chunks [128, 102]
    (102 = P*2 outputs) produce UV.T [102, 8*64] f32 in one psum bank:
    psum[:, j*64:(j+1)*64] = (matched_j @ [W_sub.T | W_obj.T]).T.
  - One f32->bf16 cast copy per group (DVE/ACT alternating), one store per
    group on the scalar queue. 48 matmuls total vs 1171 in the one-hot design.

hs and W_pred are bf16 on-chip (psum accumulates f32); U/V return as bf16,
host adds them in f32 — ~3e-3 relative error vs the f32 reference.
"""

import sys

import numpy as np

L, B, Q1, D = 6, 256, 101, 256
M, R, P = 64, 64, 51
NCORES = 8
BLOC = B // NCORES          # images per core
NB = L * BLOC               # (layer, image) blocks per core
G = 8                       # blocks per psum-bank group
NG = NB // G                # groups per core
ISPLIT = (0, 2, 6, 14, 20, 22, 24)  # input DMA boundaries, in groups
OSPLIT = {3: 0, 7: 4, 9: 8, 10: 10, 11: 11}  # out DMA: pair -> start pair
P2 = 2 * P                  # stacked U|V output features
PP = 128                    # P2 padded to full partitions (16-engine DMA)

_CACHE = {}


def _build_program():
    import concourse.bacc as bacc
    import concourse.mybir as mybir
    import concourse.tile as tile
    from contextlib import ExitStack

    f32 = mybir.dt.float32
    bf16 = mybir.dt.bfloat16
    nc = bacc.Bacc("TRN2", target_bir_lowering=False, debug=False)

    GW = G * M              # cols per group-chunk rhs tile (512)
    mt = nc.dram_tensor("mt", [128, NG * 2 * GW], bf16,
                        kind="ExternalInput").ap()
    wt = nc.dram_tensor("wt", [128, 2 * PP], bf16, kind="ExternalInput").ap()
    out = nc.dram_tensor("out", [PP, NG * GW], bf16,
                         kind="ExternalOutput").ap()

    with tile.TileContext(nc) as tc, ExitStack() as ctx:
        # static arenas (everything fits in SBUF): no pool rotation, so no
        # WAR semaphores; one tile per input DMA keeps dep tracking exact
        sb = ctx.enter_context(tc.tile_pool(name="sb", bufs=1))
        ps = ctx.enter_context(tc.tile_pool(name="ps", bufs=4, space="PSUM"))

        # const load first on the gpsimd SWDGE queue: the scalar/sync
        # (HWDGE) queues generate descriptors at only ~35/us, which would
        # gate the first LDWEIGHTS by ~3us; wt's 128 tiny lines cost the
        # SWDGE stream well under 1us
        wt_t = sb.tile([128, 2 * PP], bf16, tag="wt")
        nc.gpsimd.dma_start(out=wt_t[:], in_=wt[:])

        in_tiles = []
        for i, (a, b) in enumerate(zip(ISPLIT, ISPLIT[1:])):
            in_t = sb.tile([128, (b - a) * 2 * GW], bf16, tag=f"in{i}")
            nc.gpsimd.dma_start(out=in_t[:],
                                in_=mt[:, a * 2 * GW:b * 2 * GW])
            in_tiles.append(in_t)

        o_t = sb.tile([PP, NG * GW], bf16, tag="o")

        # HAM warm-up in the dead window while the first input chunks are
        # in flight: sustained tensor activity from ~7us pulls the clock
        # ramp (1.2 -> 2.4 GHz) several us earlier, so mid-stream matmuls
        # run at ~216ns instead of ~425ns
        wu = sb.tile([128, 512], bf16, tag="wu")
        nc.vector.memset(wu[:], 0.0)
        for _ in range(14):
            pw = ps.tile([PP, 2 * GW], f32, tag="pq")
            nc.tensor.matmul(out=pw[:, 0:512], lhsT=wu[:, 0:PP], rhs=wu[:],
                             start=True, stop=True)

        def rhs_of(g):
            i = max(k for k, a in enumerate(ISPLIT[:-1]) if ISPLIT[k] <= g)
            off = (g - ISPLIT[i]) * 2 * GW
            return in_tiles[i][:, off:off + 2 * GW]

        for p in range(NG // 2):        # pairs of groups
            pq = ps.tile([PP, 2 * GW], f32, tag="pq")
            for h in range(2):
                rhs = rhs_of(2 * p + h)
                # out cols j*64:(j+1)*64 = UV.T of block, accumulated over
                # the two 128-row d-chunks of matched.T
                nc.tensor.matmul(out=pq[:, h * GW:(h + 1) * GW],
                                 lhsT=wt_t[:, 0:PP], rhs=rhs[:, 0:GW],
                                 start=True, stop=False)
                nc.tensor.matmul(out=pq[:, h * GW:(h + 1) * GW],
                                 lhsT=wt_t[:, PP:2 * PP], rhs=rhs[:, GW:2 * GW],
                                 start=False, stop=True)
            oc = o_t[:, p * 2 * GW:(p + 1) * 2 * GW]
            if p == NG // 2 - 1:
                # split the last cast across both engines to shave the tail
                nc.vector.tensor_copy(out=oc[:, 0:GW], in_=pq[:, 0:GW])
                nc.scalar.copy(out=oc[:, GW:2 * GW], in_=pq[:, GW:2 * GW])
            elif p % 2 == 0:
                nc.vector.tensor_copy(out=oc, in_=pq[:])
            else:
                nc.scalar.copy(out=oc, in_=pq[:])
            # output DMAs on the scalar queue only (extra DMAs on the
            # gpsimd ring stall the matmul feed): big chunks early,
            # small at the end so the post-last-cast wire time is short
            if p in OSPLIT:
                s = OSPLIT[p] * 2 * GW
                nc.scalar.dma_start(out=out[:, s:(p + 1) * 2 * GW],
                                    in_=o_t[:, s:(p + 1) * 2 * GW])

    nc.compile()
    return nc


def _host_indices(src_indices, tgt_perm, relationships):
    """pos_sub, pos_obj: [L, B, R] — position in the matched list per
    relation end (the reference then maps pos -> query via src_indices)."""
    tgt = np.asarray(tgt_perm, dtype=np.int64)
    rel = np.asarray(relationships, dtype=np.int64)

    # lookup[l, b, tgt[l, b, k]] = k
    lookup = np.empty((L, B, M), dtype=np.int64)
    li = np.arange(L)[:, None, None]
    bi = np.arange(B)[None, :, None]
    lookup[li, bi, tgt] = np.broadcast_to(np.arange(M), (L, B, M))

    sub_t = np.broadcast_to(rel[None, :, :, 0], (L, B, R))
    obj_t = np.broadcast_to(rel[None, :, :, 1], (L, B, R))
    pos_sub = np.take_along_axis(lookup, sub_t, axis=2)
    pos_obj = np.take_along_axis(lookup, obj_t, axis=2)
    return pos_sub, pos_obj


def _host_prepare(hs, src_indices, tgt_perm, relationships, W_pred, b_pred):
    """Build per-core input maps (matched rows, transposed + group-packed)."""
    import ml_dtypes
    bf16 = ml_dtypes.bfloat16

    hs = np.asarray(hs, dtype=np.float32)
    src = np.asarray(src_indices, dtype=np.int64)
    W = np.asarray(W_pred, dtype=np.float32)

    # matched rows: hs[l, b, src[l, b, k], :] -> [L, B, M, D]
    matched = np.take_along_axis(hs, src[..., None], axis=2).astype(bf16)

    # W chunks: wt[:, 0:P2] = Wcat[0:128, :], wt[:, P2:] = Wcat[128:256, :]
    # where Wcat [2D? no: D x P2] hmm — Wcat[d, p] = W_sub.T | W_obj.T
    Wcat = np.concatenate([W[:, :D].T, W[:, D:].T], axis=1)    # [D, P2]
    Wcat = np.concatenate(
        [Wcat, np.zeros((2 * D // 2, PP - P2), np.float32)], axis=1)  # pad
    wt_packed = np.ascontiguousarray(
        Wcat.reshape(2, 128, PP).transpose(1, 0, 2).reshape(128, 2 * PP)
    ).astype(bf16)

    in_maps = []
    for c in range(NCORES):
        sl = slice(c * BLOC, (c + 1) * BLOC)
        # [L, BLOC, M, D] -> matched.T chunks [NB, 2, 128, M]
        mt_core = (matched[:, sl].transpose(0, 1, 3, 2)
                   .reshape(NB, 2, 128, M))
        # group-pack: [NG, G, 2, 128, M] -> [128, NG, 2, G, M]: flat col
        # order (group, chunk, block, k), partition dim first
        mt_core = mt_core.reshape(NG, G, 2, 128, M).transpose(3, 0, 2, 1, 4)
        mt_core = np.ascontiguousarray(mt_core.reshape(128, NG * 2 * G * M))
        in_maps.append({"mt": mt_core, "wt": wt_packed})
    return in_maps


def kernel(hs, src_indices, tgt_perm, relationships, W_pred, b_pred):
    if "concourse" not in sys.modules:
        try:
            import concourse  # noqa: F401
        except ImportError:
            sys.path.insert(0, "/opt/trn_rl_repo")
    from concourse import bass_utils

    in_maps = _host_prepare(hs, src_indices, tgt_perm, relationships,
                            W_pred, b_pred)
    if "nc" not in _CACHE:
        _CACHE["nc"] = _build_program()
    nc = _CACHE["nc"]

    res = bass_utils.run_bass_kernel_spmd(nc, in_maps, list(range(NCORES)))

    # reassemble U, V: out [NG, P2, G*M] -> per block [M, P2] = [U | V]
    uv_cores = []
    for c in range(NCORES):
        o = np.asarray(res.results[c]["out"], dtype=np.float32)
        # [P2, NG*G*M] cols (g, j, k) -> [L, BLOC, M, P2]
        o = (o.reshape(PP, NG, G, M)[:P2].transpose(1, 2, 3, 0)
             .reshape(L, BLOC, M, P2))
        uv_cores.append(o)
    uv = np.concatenate(uv_cores, axis=1)                      # [L, B, M, P2]

    pos_sub, pos_obj = _host_indices(src_indices, tgt_perm, relationships)
    U = uv[..., :P]                                            # [L, B, M, P]
    V = uv[..., P:]
    b = np.asarray(b_pred, dtype=np.float32)
    logits = (np.take_along_axis(U, pos_sub[..., None], axis=2)
              + np.take_along_axis(V, pos_obj[..., None], axis=2) + b)
    return np.ascontiguousarray(logits, dtype=np.float32)


# revision 30
# speedup vs baseline: 1.0074x; 1.0074x over previous
"""DETR scene-graph predicate head on 8 Trainium2 NeuronCores.

Math: logits[l,b,r,:] = concat(hs[l,b,q_sub], hs[l,b,q_obj]) @ W_pred.T + b_pred
where q_sub/q_obj come from (tgt_perm inverse, relationships, src_indices) —
pure integer index math, done on host.

Key factorization: split W_pred [P, 2D] into W_sub|W_obj [P, D] halves and
compute, per (layer, image) block, U = matched @ W_sub.T and V = matched @
W_obj.T over the M=64 *matched* queries (matched = hs[l,b,src_indices[l,b]]).
Then logits[r] = U[pos_sub[r]] + V[pos_obj[r]] + b — a cheap index-select the
host applies while unsharding. This cuts device HBM traffic 3x vs shipping
hs+one-hots (each matched row is read once, not once per relation end) and
turns the whole kernel into one W-stationary GEMM.

Device layout (batch axis sharded 8 ways; L*B/8 = 192 blocks/core):
  - Host sends matched.T per block as two 128-row chunks [2, 128, 64] bf16,
    packed 8 blocks/group side-by-side -> rhs tiles [128, 512], two groups per
    DMA [128, 2048] (4 KB/partition lines keep all 16 SDMA engines busy on the
    gpsimd SWDGE queue).
  - Per group: 2 accumulating matmuls with stationary W chunks [128, 102]
    (102 = P*2 outputs) produce UV.T [102, 8*64] f32 in one psum bank:
    psum[:, j*64:(j+1)*64] = (matched_j @ [W_sub.T | W_obj.T]).T.
  - One f32->bf16 cast copy per group (DVE/ACT alternating), one store per
    group on the scalar queue. 48 matmuls total vs 1171 in the one-hot design.

hs and W_pred are bf16 on-chip (psum accumulates f32); U/V return as bf16,
host adds them in f32 — ~3e-3 relative error vs the f32 reference.
"""

import sys

import numpy as np

L, B, Q1, D = 6, 256, 101, 256
M, R, P = 64, 64, 51
NCORES = 8
BLOC = B // NCORES          # images per core
NB = L * BLOC               # (layer, image) blocks per core
G = 8                       # blocks per psum-bank group
NG = NB // G                # groups per core
ISPLIT = (0, 2, 6, 14, 20, 22, 24)  # input DMA boundaries, in groups
OSPLIT = {3: 0, 7: 4, 9: 8, 10: 10, 11: 11}  # out DMA: pair -> start pair
P2 = 2 * P                  # stacked U|V output features
PP = 128                    # P2 padded to full partitions (16-engine DMA)

_CACHE = {}


def _build_program():
    import concourse.bacc as bacc
    import concourse.mybir as mybir
    import concourse.tile as tile
    from contextlib import ExitStack

    f32 = mybir.dt.float32
    bf16 = mybir.dt.bfloat16
    nc = bacc.Bacc("TRN2", target_bir_lowering=False, debug=False)

    GW = G * M              # cols per group-chunk rhs tile (512)
    mt = nc.dram_tensor("mt", [128, NG * 2 * GW], bf16,
                        kind="ExternalInput").ap()
    wt = nc.dram_tensor("wt", [128, 2 * PP], bf16, kind="ExternalInput").ap()
    out = nc.dram_tensor("out", [PP, NG * GW], bf16,
                         kind="ExternalOutput").ap()

    with tile.TileContext(nc) as tc, ExitStack() as ctx:
        # static arenas (everything fits in SBUF): no pool rotation, so no
        # WAR semaphores; one tile per input DMA keeps dep tracking exact
        sb = ctx.enter_context(tc.tile_pool(name="sb", bufs=1))
        ps = ctx.enter_context(tc.tile_pool(name="ps", bufs=4, space="PSUM"))

        # const load first on the gpsimd SWDGE queue: the scalar/sync
        # (HWDGE) queues generate descriptors at only ~35/us, which would
        # gate the first LDWEIGHTS by ~3us; wt's 128 tiny lines cost the
        # SWDGE stream well under 1us
        wt_t = sb.tile([128, 2 * PP], bf16, tag="wt")
        nc.gpsimd.dma_start(out=wt_t[:], in_=wt[:])

        in_tiles = []
        for i, (a, b) in enumerate(zip(ISPLIT, ISPLIT[1:])):
            in_t = sb.tile([128, (b - a) * 2 * GW], bf16, tag=f"in{i}")
            nc.gpsimd.dma_start(out=in_t[:],
                                in_=mt[:, a * 2 * GW:b * 2 * GW])
            in_tiles.append(in_t)

        o_t = sb.tile([PP, NG * GW], bf16, tag="o")

        # HAM warm-up in the dead window while the first input chunks are
        # in flight: sustained tensor activity from ~7us pulls the clock
        # ramp (1.2 -> 2.4 GHz) several us earlier, so mid-stream matmuls
        # run at ~216ns instead of ~425ns
        wu = sb.tile([128, 512], bf16, tag="wu")
        nc.vector.memset(wu[:], 0.0)
        for _ in range(14):
            pw = ps.tile([PP, 2 * GW], f32, tag="pq")
            nc.tensor.matmul(out=pw[:, 0:512], lhsT=wu[:, 0:PP], rhs=wu[:],
                             start=True, stop=True)

        def rhs_of(g):
            i = max(k for k, a in enumerate(ISPLIT[:-1]) if ISPLIT[k] <= g)
            off = (g - ISPLIT[i]) * 2 * GW
            return in_tiles[i][:, off:off + 2 * GW]

        for p in range(NG // 2):        # pairs of groups
            pq = ps.tile([PP, 2 * GW], f32, tag="pq")
            for h in range(2):
                rhs = rhs_of(2 * p + h)
                # out cols j*64:(j+1)*64 = UV.T of block, accumulated over
                # the two 128-row d-chunks of matched.T
                nc.tensor.matmul(out=pq[:, h * GW:(h + 1) * GW],
                                 lhsT=wt_t[:, 0:PP], rhs=rhs[:, 0:GW],
                                 start=True, stop=False)
                nc.tensor.matmul(out=pq[:, h * GW:(h + 1) * GW],
                                 lhsT=wt_t[:, PP:2 * PP], rhs=rhs[:, GW:2 * GW],
                                 start=False, stop=True)
            oc = o_t[:, p * 2 * GW:(p + 1) * 2 * GW]
            # alternate cast engines, but keep the LAST cast on vector:
            # scalar also issues the output DMA triggers, and serializing
            # cast11 behind them delays the final store by >1us
            if p % 2 == 0 or p == NG // 2 - 1:
                nc.vector.tensor_copy(out=oc, in_=pq[:])
            else:
                nc.scalar.copy(out=oc, in_=pq[:])
            # output DMAs on the scalar queue only (extra DMAs on the
            # gpsimd ring stall the matmul feed): big chunks early,
            # small at the end so the post-last-cast wire time is short
            if p in OSPLIT:
                s = OSPLIT[p] * 2 * GW
                nc.scalar.dma_start(out=out[:, s:(p + 1) * 2 * GW],
                                    in_=o_t[:, s:(p + 1) * 2 * GW])

    nc.compile()
    return nc


def _host_indices(src_indices, tgt_perm, relationships):
    """pos_sub, pos_obj: [L, B, R] — position in the matched list per
    relation end (the reference then maps pos -> query via src_indices)."""
    tgt = np.asarray(tgt_perm, dtype=np.int64)
    rel = np.asarray(relationships, dtype=np.int64)

    # lookup[l, b, tgt[l, b, k]] = k
    lookup = np.empty((L, B, M), dtype=np.int64)
    li = np.arange(L)[:, None, None]
    bi = np.arange(B)[None, :, None]
    lookup[li, bi, tgt] = np.broadcast_to(np.arange(M), (L, B, M))

    sub_t = np.broadcast_to(rel[None, :, :, 0], (L, B, R))
    obj_t = np.broadcast_to(rel[None, :, :, 1], (L, B, R))
    pos_sub = np.take_along_axis(lookup, sub_t, axis=2)
    pos_obj = np.take_along_axis(lookup, obj_t, axis=2)
    return pos_sub, pos_obj


def _host_prepare(hs, src_indices, tgt_perm, relationships, W_pred, b_pred):
    """Build per-core input maps (matched rows, transposed + group-packed)."""
    import ml_dtypes
    bf16 = ml_dtypes.bfloat16

    hs = np.asarray(hs, dtype=np.float32)
    src = np.asarray(src_indices, dtype=np.int64)
    W = np.asarray(W_pred, dtype=np.float32)

    # matched rows: hs[l, b, src[l, b, k], :] -> [L, B, M, D]
    matched = np.take_along_axis(hs, src[..., None], axis=2).astype(bf16)

    # W chunks: wt[:, 0:P2] = Wcat[0:128, :], wt[:, P2:] = Wcat[128:256, :]
    # where Wcat [2D? no: D x P2] hmm — Wcat[d, p] = W_sub.T | W_obj.T
    Wcat = np.concatenate([W[:, :D].T, W[:, D:].T], axis=1)    # [D, P2]
    Wcat = np.concatenate(
        [Wcat, np.zeros((2 * D // 2, PP - P2), np.float32)], axis=1)  # pad
    wt_packed = np.ascontiguousarray(
        Wcat.reshape(2, 128, PP).transpose(1, 0, 2).reshape(128, 2 * PP)
    ).astype(bf16)

    in_maps = []
    for c in range(NCORES):
        sl = slice(c * BLOC, (c + 1) * BLOC)
        # [L, BLOC, M, D] -> matched.T chunks [NB, 2, 128, M]
        mt_core = (matched[:, sl].transpose(0, 1, 3, 2)
                   .reshape(NB, 2, 128, M))
        # group-pack: [NG, G, 2, 128, M] -> [128, NG, 2, G, M]: flat col
        # order (group, chunk, block, k), partition dim first
        mt_core = mt_core.reshape(NG, G, 2, 128, M).transpose(3, 0, 2, 1, 4)
        mt_core = np.ascontiguousarray(mt_core.reshape(128, NG * 2 * G * M))
        in_maps.append({"mt": mt_core, "wt": wt_packed})
    return in_maps


def kernel(hs, src_indices, tgt_perm, relationships, W_pred, b_pred):
    if "concourse" not in sys.modules:
        try:
            import concourse  # noqa: F401
        except ImportError:
            sys.path.insert(0, "/opt/trn_rl_repo")
    from concourse import bass_utils

    in_maps = _host_prepare(hs, src_indices, tgt_perm, relationships,
                            W_pred, b_pred)
    if "nc" not in _CACHE:
        _CACHE["nc"] = _build_program()
    nc = _CACHE["nc"]

    res = bass_utils.run_bass_kernel_spmd(nc, in_maps, list(range(NCORES)))

    # reassemble U, V: out [NG, P2, G*M] -> per block [M, P2] = [U | V]
    uv_cores = []
    for c in range(NCORES):
        o = np.asarray(res.results[c]["out"], dtype=np.float32)
        # [P2, NG*G*M] cols (g, j, k) -> [L, BLOC, M, P2]
        o = (o.reshape(PP, NG, G, M)[:P2].transpose(1, 2, 3, 0)
             .reshape(L, BLOC, M, P2))
        uv_cores.append(o)
    uv = np.concatenate(uv_cores, axis=1)                      # [L, B, M, P2]

    pos_sub, pos_obj = _host_indices(src_indices, tgt_perm, relationships)
    U = uv[..., :P]                                            # [L, B, M, P]
    V = uv[..., P:]
    b = np.asarray(b_pred, dtype=np.float32)
    logits = (np.take_along_axis(U, pos_sub[..., None], axis=2)
              + np.take_along_axis(V, pos_obj[..., None], axis=2) + b)
    return np.ascontiguousarray(logits, dtype=np.float32)
